# revision 23
# baseline (speedup 1.0000x reference)
"""Multi-head causal attention on 8 Trainium2 NeuronCores.

Sharding: data-parallel over batch (4) x tensor-parallel over heads (2 groups
of 8 heads). Each core computes a partial output [T, C] for one batch element
using its 8 heads; the host sums the two partials per batch element (the
"all-reduce after out_proj" done during unshard).

Design notes (HW exec ~275us vs 394us baseline):
  - Inputs host-pre-arranged so every DMA is contiguous per partition; the
    first-needed weights go on the scalar DMA queue and x chunk 0 is split
    into four independent quarter-tiles so the first matmul starts ~13us.
  - One interleaved instruction stream: projection work units for token
    chunk t+1, the out-projection for chunk j-1, and per-pair softmax
    normalization are emitted between (and sparsely inside) attention
    head-pairs of chunk j via a deadline-guarded micro-op queue, so the PE
    never idles long enough for the HAM clock gate to re-throttle. The
    deadline guard force-emits any deferred producer right before its
    consumer, so correctness never depends on the injection cadence.
  - Causal staircase computed at partial width: for key block kb of query
    chunk j only queries >= kb*128 are computed (saves ~25% of score/AV
    matmul columns and exp columns); only the leading 128 columns of a
    diagonal block need the triangular mask multiply.
  - Denominators (ones-row of the augmented V matmul) are staged through a
    1-partition tile, spread to 2 partitions by a tiny DMA on the otherwise
    idle GpSimd queue (keeping them off the busy sync queue), reciprocal'd
    with the 1-op ~51-ULP approx reciprocal, broadcast via a K=2 bf16
    matmul, and applied in-place to attn_outT one pair later so the PE
    never waits on the chain.
  - Final chunk's out-projection runs cc0-2 partial chains in the freed
    score-PSUM slots concurrently with the last attention pair; only the
    cc3 matmuls + cast + DMA trail the last normalization.
  - Output written bf16 (halves writeback); host upcasts and sums partials.

Per-core layouts (partition dim first):
  qt/kt/aot [128, 4, 2048]: partition = (head%2)*64 + d, dim1 = head//2 (pair)
  vaug [128, 16, 8, 65] bf16: partition = key-in-block, ones-augmented col 64
  scores^T per (pair, kb): psum [128, 2, 512] = key x (half, query)
"""

import numpy as np
import ml_dtypes

_BF = ml_dtypes.bfloat16

import concourse.bass as bass
import concourse.bacc as bacc
import concourse.mybir as mybir
import concourse.tile as tile
from concourse import bass_utils

F32 = mybir.dt.float32
F32R = mybir.dt.float32r
BF16 = mybir.dt.bfloat16

B, T, C = 4, 2048, 1024
H, Dh = 16, 64
G = 2                 # head groups (tensor parallel)
HPG = H // G          # 8 heads per group
GC = HPG * Dh         # group channels = 512
N_CORES = 8
TC = 512              # token chunk
KB = 128              # key block
N_TC = T // TC        # 4
N_KB = T // KB        # 16
N_CC = C // 128       # contraction chunks over C = 8
N_GCB = GC // 128     # head pairs = 4


def build_program():
    nc = bacc.Bacc("TRN2", target_bir_lowering=False, debug=False)

    xT = nc.dram_tensor("xT", [N_TC, 128, N_CC, TC], BF16, kind="ExternalInput").ap()
    wq = nc.dram_tensor("wq", [128, N_GCB, N_CC, 128], BF16, kind="ExternalInput").ap()
    wk = nc.dram_tensor("wk", [128, N_GCB, N_CC, 128], BF16, kind="ExternalInput").ap()
    wv = nc.dram_tensor("wv", [128, N_CC, GC], BF16, kind="ExternalInput").ap()
    wo = nc.dram_tensor("wo", [128, N_GCB, C], BF16, kind="ExternalInput").ap()
    masks = nc.dram_tensor("masks", [KB, 2, KB], BF16, kind="ExternalInput").ap()
    sel_in = nc.dram_tensor("sel", [2, 128], BF16, kind="ExternalInput").ap()
    out = nc.dram_tensor("out", [T, C], BF16, kind="ExternalOutput").ap()

    EXP = mybir.ActivationFunctionType.Exp

    with tile.TileContext(nc) as tc:
        with (
            tc.tile_pool(name="persist", bufs=1) as pp,
            tc.tile_pool(name="xp", bufs=2) as xp,
            tc.tile_pool(name="pr_pool", bufs=4) as prp,
            tc.tile_pool(name="ot_pool", bufs=4) as otp,
            tc.tile_pool(name="dn_pool", bufs=2) as dnp,
            tc.tile_pool(name="sc_psum", bufs=2, space="PSUM") as scp,
            tc.tile_pool(name="av_psum", bufs=1, space="PSUM") as avp,
            tc.tile_pool(name="ps_psum", bufs=2, space="PSUM") as psp,
        ):
            qt = pp.tile([128, N_GCB, T], BF16)
            kt = pp.tile([128, N_GCB, T], BF16)
            vaug = pp.tile([128, N_KB, HPG, Dh + 1], BF16)
            aot = pp.tile([128, N_GCB, T], BF16)
            msk = pp.tile([KB, 2, KB], BF16)
            sel = pp.tile([2, 128], BF16)
            wqs = pp.tile([128, N_GCB, N_CC, 128], BF16)
            wks = pp.tile([128, N_GCB, N_CC, 128], BF16)
            wvs = pp.tile([128, N_CC, GC], BF16)
            wos = pp.tile([128, N_GCB, C], BF16)

            # ---- input DMAs: x chunk 0 on the scalar queue, weights on ---
            # ---- sync, so desc-gen and transfers overlap -----------------
            xts = [None] * N_TC

            def dma_x(t, eng=None):
                xts[t] = xp.tile([128, N_CC, TC], BF16, tag="xt", name=f"xt{t}")
                (eng or nc.sync).dma_start(xts[t][:], xT[t])

            # chunk-0 x split into eight independent slab tiles so the
            # first projection matmuls start as soon as the first slab lands
            x0q = []
            for q in range(N_CC):
                x0t = xp.tile([128, 1, TC], BF16, tag=f"x0q{q}", name=f"x0q{q}", bufs=1)
                x0q.append(x0t)
            nc.scalar.dma_start(wqs[:, 0, 0:2], wq[:, 0, 0:2])
            nc.scalar.dma_start(wqs[:, 0, 2:], wq[:, 0, 2:])
            for oc in range(1, N_GCB):
                nc.scalar.dma_start(wqs[:, oc], wq[:, oc])
            for q in range(N_CC):
                nc.sync.dma_start(x0q[q][:], xT[0][:, q:q + 1])
            for oc in range(N_GCB):
                nc.sync.dma_start(wks[:, oc], wk[:, oc])
            nc.sync.dma_start(wvs[:], wv)
            nc.sync.dma_start(msk[:], masks)
            nc.sync.dma_start(sel[:], sel_in)

            def xslice(t, kc):
                if t == 0:
                    return x0q[kc][:, 0]
                return xts[t][:, kc]
            nc.vector.memset(vaug[:, :, :, Dh:], 1.0)

            # ---- qkv projection work units for token chunk t -------------
            def unit_qk(t, oc, w_s, dst):
                ps = psp.tile([128, TC], F32, tag="ps", name="pjq")
                for kc in range(N_CC):
                    nc.tensor.matmul(
                        ps[:], w_s[:, oc, kc], xslice(t, kc),
                        start=(kc == 0), stop=(kc == N_CC - 1),
                    )
                nc.vector.tensor_copy(dst[:, oc, t * TC:(t + 1) * TC], ps[:])

            def unit_v(t, tb):
                ps = psp.tile([128, GC], F32, tag="ps", name="pjv")
                for kc in range(N_CC):
                    nc.tensor.matmul(
                        ps[:], xslice(t, kc)[:, tb * 128:(tb + 1) * 128],
                        wvs[:, kc],
                        start=(kc == 0), stop=(kc == N_CC - 1),
                    )
                nc.vector.tensor_copy(
                    vaug[:, t * 4 + tb, :, :Dh],
                    ps.rearrange("p (h d) -> p h d", h=HPG),
                )

            def phase2_units(t, q_first=False):
                us = []
                if q_first:
                    for oc in range(N_GCB):
                        us.append(lambda oc=oc: unit_qk(t, oc, wqs, qt))
                    for oc in range(N_GCB):
                        us.append(lambda oc=oc: unit_qk(t, oc, wks, kt))
                else:
                    for oc in range(N_GCB):
                        us.append(lambda oc=oc: unit_qk(t, oc, wqs, qt))
                        us.append(lambda oc=oc: unit_qk(t, oc, wks, kt))
                for tb in range(4):
                    us.append(lambda tb=tb: unit_v(t, tb))
                return us

            # ---- micro-op decompositions for fine-grained interleave -----
            def micro_qk(t, oc, w_s, dst):
                st = {}
                def a():
                    st["ps"] = psp.tile([128, TC], F32, tag="ps", name="pjq")
                    for kc in range(4):
                        nc.tensor.matmul(
                            st["ps"][:], w_s[:, oc, kc], xts[t][:, kc],
                            start=(kc == 0), stop=False,
                        )
                def b():
                    for kc in range(4, N_CC):
                        nc.tensor.matmul(
                            st["ps"][:], w_s[:, oc, kc], xts[t][:, kc],
                            start=False, stop=(kc == N_CC - 1),
                        )
                    nc.vector.tensor_copy(
                        dst[:, oc, t * TC:(t + 1) * TC], st["ps"][:]
                    )
                return [a, b]

            def micro_v(t, tb):
                st = {}
                def a():
                    st["ps"] = psp.tile([128, GC], F32, tag="ps", name="pjv")
                    for kc in range(4):
                        nc.tensor.matmul(
                            st["ps"][:],
                            xts[t][:, kc, tb * 128:(tb + 1) * 128],
                            wvs[:, kc], start=(kc == 0), stop=False,
                        )
                def b():
                    for kc in range(4, N_CC):
                        nc.tensor.matmul(
                            st["ps"][:],
                            xts[t][:, kc, tb * 128:(tb + 1) * 128],
                            wvs[:, kc], start=False, stop=(kc == N_CC - 1),
                        )
                    nc.vector.tensor_copy(
                        vaug[:, t * 4 + tb, :, :Dh],
                        st["ps"].rearrange("p (h d) -> p h d", h=HPG),
                    )
                return [a, b]

            def micro_tb(tb):
                st = {}
                def half(oc, lo):
                    if oc == 0 and lo == 0:
                        ots[tb % 4] = otp.tile([128, C], BF16, tag="ot", name="ot")
                    if lo == 0:
                        st["ps"] = psp.tile([128, TC], F32, tag="ps", name="op")
                    for cc in range(lo, lo + 2):
                        nc.tensor.matmul(
                            st["ps"][:],
                            aot[:, cc, tb * 128:(tb + 1) * 128],
                            wos[:, cc, oc * TC:(oc + 1) * TC],
                            start=(cc == 0), stop=(cc == N_GCB - 1),
                        )
                    if lo == 2:
                        nc.vector.tensor_copy(
                            ots[tb % 4][:, oc * TC:(oc + 1) * TC], st["ps"][:]
                        )
                        if oc == 1:
                            nc.sync.dma_start(
                                out[tb * 128:(tb + 1) * 128], ots[tb % 4][:]
                            )
                return [lambda oc=oc, lo=lo: half(oc, lo)
                        for oc in range(2) for lo in (0, 2)]

            ots = [None] * 4

            rec_t = [None] * (N_TC * N_GCB)
            from collections import deque
            dq = deque()   # (deadline (j,p,kb), fn) — deadlines non-decreasing
            fq = deque()   # free micros (no ordering constraint)

            def run_due(pos):
                while dq and dq[0][0] <= pos:
                    dq.popleft()[1]()

            def inject():
                if dq:
                    dq.popleft()[1]()
                elif fq:
                    fq.popleft()()

            # ---- attention + fused normalize for (chunk j, head pair p) --
            def attn_pair(j, p):
                av = avp.tile([Dh + 1, 2, TC], F32, tag="av", name="av")
                nkb = 4 * j + 4
                for kb in range(nkb):
                    run_due((j, p, kb))
                    off = KB * (kb - 4 * j) if kb >= 4 * j else 0
                    sc = scp.tile([128, 2, TC], F32, tag="sc", name="sc")
                    for half in range(2):
                        p0 = half * Dh
                        nc.tensor.matmul(
                            sc[:, half, off:],
                            kt[p0:p0 + Dh, p, kb * KB:(kb + 1) * KB],
                            qt[p0:p0 + Dh, p, j * TC + off:(j + 1) * TC],
                            start=True, stop=True,
                        )
                    pr = prp.tile([128, 2, TC], BF16, tag="pr", name="pr")
                    nc.scalar.activation(pr[:, :, off:], sc[:, :, off:], EXP)
                    if kb >= 4 * j:
                        nc.vector.tensor_mul(
                            pr[:, :, off:off + KB], pr[:, :, off:off + KB],
                            msk[:],
                        )
                    if kb % 6 == 1:
                        inject()
                    for half in range(2):
                        nc.tensor.matmul(
                            av[:, half, off:],
                            vaug[:, kb, 2 * p + half],
                            pr[:, half, off:],
                            start=(kb == 0), stop=(kb == nkb - 1),
                            skip_group_check=True,
                        )
                # denominators first (they head the recip->norm chain),
                # then the attn-output drains
                dn = dnp.tile([1, 2, TC], F32, tag="dn", name="dn", bufs=4)
                if (j, p) == (N_TC - 1, N_GCB - 1):
                    nc.scalar.copy(dn[:], av[Dh:Dh + 1])
                else:
                    nc.vector.tensor_copy(dn[:], av[Dh:Dh + 1])
                for half in range(2):
                    p0 = half * Dh
                    nc.vector.tensor_copy(
                        aot[p0:p0 + Dh, p, j * TC:(j + 1) * TC], av[:Dh, half]
                    )
                dc = dnp.tile([2, TC], F32, tag="dc", name="dc", bufs=4)
                nc.gpsimd.dma_start(dc[:], dn[0:1])
                rc = dnp.tile([2, TC], F32, tag="rc", name="rc", bufs=4)
                nc.vector.reciprocal_approx_fast(rc[:], dc[:])
                rcb = dnp.tile([2, TC], BF16, tag="rcb", name="rcb", bufs=4)
                rec_t[4 * j + p] = rcb
                nc.vector.tensor_copy(rcb[:], rc[:])

            # ---- softmax divide for (chunk j, head pair p), scheduled ----
            # ---- one pair later so the PE never waits on the recip chain -
            def norm_pair(j, p):
                rc = rec_t[4 * j + p]  # bf16 reciprocal rows
                bc = psp.tile([128, TC], F32, tag="ps", name="bc")
                nc.tensor.matmul(
                    bc[:], sel[:], rc[:],
                    start=True, stop=True,
                )
                nc.vector.tensor_mul(
                    aot[:, p, j * TC:(j + 1) * TC],
                    aot[:, p, j * TC:(j + 1) * TC],
                    bc[:],
                )

            # ---- out-projection for one token block ----------------------
            def tail_tb(tb):
                ot = otp.tile([128, C], BF16, tag="ot", name="ot")
                for oc in range(2):
                    ps = psp.tile([128, TC], F32, tag="ps", name="op")
                    for cc in range(N_GCB):
                        nc.tensor.matmul(
                            ps[:],
                            aot[:, cc, tb * 128:(tb + 1) * 128],
                            wos[:, cc, oc * TC:(oc + 1) * TC],
                            start=(cc == 0), stop=(cc == N_GCB - 1),
                        )
                    if oc == 0:
                        nc.scalar.copy(ot[:, oc * TC:(oc + 1) * TC], ps[:])
                    else:
                        nc.vector.tensor_copy(ot[:, oc * TC:(oc + 1) * TC], ps[:])
                    nc.sync.dma_start(
                        out[tb * 128:(tb + 1) * 128, oc * TC:(oc + 1) * TC],
                        ot[:, oc * TC:(oc + 1) * TC],
                    )

            # ---- interleaved schedule ------------------------------------
            for u in phase2_units(0, q_first=True):
                u()

            for j in range(N_TC):
                t = j + 1
                if t < N_TC:
                    dma_x(t)
                if j == 0:
                    nc.sync.dma_start(wos[:], wo)
                if j >= 1:
                    for tb in range(4 * (j - 1), 4 * j):
                        fq.extend(micro_tb(tb))
                if t < N_TC:
                    for m in micro_qk(t, 0, wqs, qt):
                        dq.append(((t, 0, 0), m))
                    for m in micro_qk(t, 0, wks, kt):
                        dq.append(((t, 0, 4 * t), m))
                    for tb in range(4):
                        for m in micro_v(t, tb):
                            dq.append(((t, 0, 4 * t + tb), m))
                    for oc in range(1, N_GCB):
                        for m in micro_qk(t, oc, wqs, qt):
                            dq.append(((t, oc, 0), m))
                        for m in micro_qk(t, oc, wks, kt):
                            dq.append(((t, oc, 4 * t), m))
                for p in range(N_GCB):
                    attn_pair(j, p)
                    if p >= 1:
                        norm_pair(j, p - 1)
                    inject()
                    inject()
                    inject()
                if j < N_TC - 1:
                    fq.append(lambda j=j: norm_pair(j, 3))
            while dq or fq:
                inject()
            # final chunk: out-projection cc0-2 partial chains overlap the
            # last attention pair's exp stream; only the cc3 matmuls (and
            # the cast+DMA) wait on the last pair's normalization
            last = N_TC - 1
            chains = []

            def part_a(tb, cps):
                for oc in range(2):
                    for cc in range(3):
                        nc.tensor.matmul(
                            cps[oc][:],
                            aot[:, cc, tb * 128:(tb + 1) * 128],
                            wos[:, cc, oc * TC:(oc + 1) * TC],
                            start=(cc == 0), stop=False,
                            skip_group_check=True,
                        )
                chains.append((tb, cps))

            for tb in range(4 * last, 4 * last + 2):
                hold = scp.tile([128, 2, TC], F32, tag="sc", name="opsc")
                part_a(tb, [hold[:, 0], hold[:, 1]])
            norm_pair(last, 3)
            part_a(4 * last + 2, [
                psp.tile([128, TC], F32, tag="ps", name="opa"),
                psp.tile([128, TC], F32, tag="ps", name="opb"),
            ])
            for tb, cps in chains:
                ot = otp.tile([128, C], BF16, tag="ot", name="ot")
                for oc in range(2):
                    nc.tensor.matmul(
                        cps[oc][:],
                        aot[:, 3, tb * 128:(tb + 1) * 128],
                        wos[:, 3, oc * TC:(oc + 1) * TC],
                        start=False, stop=True,
                        skip_group_check=True,
                    )
                    eng = nc.scalar if oc == 0 else nc.vector
                    if oc == 0:
                        nc.scalar.copy(ot[:, oc * TC:(oc + 1) * TC], cps[oc][:])
                    else:
                        nc.vector.tensor_copy(
                            ot[:, oc * TC:(oc + 1) * TC], cps[oc][:]
                        )
                    nc.sync.dma_start(
                        out[tb * 128:(tb + 1) * 128, oc * TC:(oc + 1) * TC],
                        ot[:, oc * TC:(oc + 1) * TC],
                    )
            tail_tb(4 * last + 3)

    nc.compile()
    return nc


_CACHE = {}


def _make_masks():
    m = np.zeros((KB, 2, KB), np.float32)
    for dk in range(KB):
        m[dk, :, dk:] = 1.0
    return m.astype(_BF)


def _make_sel():
    s = np.zeros((2, 128), np.float32)
    for m in range(128):
        s[m // Dh, m] = 1.0
    return s.astype(_BF)


def make_in_maps(x, W_qkv, W_out):
    masks = _make_masks()
    sel = _make_sel()
    in_maps = []
    for core in range(N_CORES):
        b, g = divmod(core, G)
        cs = slice(g * GC, (g + 1) * GC)
        xt_arr = np.ascontiguousarray(
            x[b].T.reshape(N_CC, 128, N_TC, TC).transpose(2, 1, 0, 3)
        ).astype(_BF)
        wq_l = np.ascontiguousarray(
            (W_qkv[:, cs] * 0.125)
            .reshape(N_CC, 128, N_GCB, 2, Dh)
            .transpose(1, 2, 0, 3, 4)
            .reshape(128, N_GCB, N_CC, 128)
        ).astype(_BF)
        wk_l = np.ascontiguousarray(
            W_qkv[:, C + g * GC:C + (g + 1) * GC]
            .reshape(N_CC, 128, N_GCB, 2, Dh)
            .transpose(1, 2, 0, 3, 4)
            .reshape(128, N_GCB, N_CC, 128)
        ).astype(_BF)
        wv_l = np.ascontiguousarray(
            W_qkv[:, 2 * C + g * GC:2 * C + (g + 1) * GC]
            .reshape(N_CC, 128, GC)
            .transpose(1, 0, 2)
        ).astype(_BF)
        wo_l = np.ascontiguousarray(
            W_out[cs, :]
            .reshape(N_GCB, 2, Dh, C)
            .transpose(1, 2, 0, 3)
            .reshape(128, N_GCB, C)
        ).astype(_BF)
        in_maps.append({
            "xT": xt_arr,
            "wq": wq_l,
            "wk": wk_l,
            "wv": wv_l,
            "wo": wo_l,
            "masks": masks,
            "sel": sel,
        })
    return in_maps


def kernel(x, W_qkv, W_out):
    x = np.ascontiguousarray(np.asarray(x, dtype=np.float32))
    W_qkv = np.asarray(W_qkv, dtype=np.float32)
    W_out = np.asarray(W_out, dtype=np.float32)

    if "nc" not in _CACHE:
        _CACHE["nc"] = build_program()
    nc = _CACHE["nc"]

    in_maps = make_in_maps(x, W_qkv, W_out)
    res = bass_utils.run_bass_kernel_spmd(nc, in_maps, core_ids=list(range(N_CORES)))

    out = np.empty((B, T, C), np.float32)
    for b in range(B):
        acc = res.results[G * b]["out"].astype(np.float32)
        for g in range(1, G):
            acc = acc + res.results[G * b + g]["out"].astype(np.float32)
        out[b] = acc
    return out


# revision 24
# speedup vs baseline: 1.0046x; 1.0046x over previous
"""Multi-head causal attention on 8 Trainium2 NeuronCores.

Sharding: data-parallel over batch (4) x tensor-parallel over heads (2 groups
of 8 heads). Each core computes a partial output [T, C] for one batch element
using its 8 heads; the host sums the two partials per batch element (the
"all-reduce after out_proj" done during unshard).

Design notes (HW exec ~275us vs 394us baseline):
  - Inputs host-pre-arranged so every DMA is contiguous per partition; the
    first-needed weights go on the scalar DMA queue and x chunk 0 is split
    into four independent quarter-tiles so the first matmul starts ~13us.
  - One interleaved instruction stream: projection work units for token
    chunk t+1, the out-projection for chunk j-1, and per-pair softmax
    normalization are emitted between (and sparsely inside) attention
    head-pairs of chunk j via a deadline-guarded micro-op queue, so the PE
    never idles long enough for the HAM clock gate to re-throttle. The
    deadline guard force-emits any deferred producer right before its
    consumer, so correctness never depends on the injection cadence.
  - Causal staircase computed at partial width: for key block kb of query
    chunk j only queries >= kb*128 are computed (saves ~25% of score/AV
    matmul columns and exp columns); only the leading 128 columns of a
    diagonal block need the triangular mask multiply.
  - Denominators (ones-row of the augmented V matmul) are staged through a
    1-partition tile, spread to 2 partitions by a tiny DMA on the otherwise
    idle GpSimd queue (keeping them off the busy sync queue), reciprocal'd
    with the 1-op ~51-ULP approx reciprocal, broadcast via a K=2 bf16
    matmul, and applied in-place to attn_outT one pair later so the PE
    never waits on the chain.
  - Final chunk's out-projection runs cc0-2 partial chains in the freed
    score-PSUM slots concurrently with the last attention pair; only the
    cc3 matmuls + cast + DMA trail the last normalization.
  - Output written bf16 (halves writeback); host upcasts and sums partials.

Per-core layouts (partition dim first):
  qt/kt/aot [128, 4, 2048]: partition = (head%2)*64 + d, dim1 = head//2 (pair)
  vaug [128, 16, 8, 65] bf16: partition = key-in-block, ones-augmented col 64
  scores^T per (pair, kb): psum [128, 2, 512] = key x (half, query)
"""

import numpy as np
import ml_dtypes

_BF = ml_dtypes.bfloat16

import concourse.bass as bass
import concourse.bacc as bacc
import concourse.mybir as mybir
import concourse.tile as tile
from concourse import bass_utils

F32 = mybir.dt.float32
F32R = mybir.dt.float32r
BF16 = mybir.dt.bfloat16

B, T, C = 4, 2048, 1024
H, Dh = 16, 64
G = 2                 # head groups (tensor parallel)
HPG = H // G          # 8 heads per group
GC = HPG * Dh         # group channels = 512
N_CORES = 8
TC = 512              # token chunk
KB = 128              # key block
N_TC = T // TC        # 4
N_KB = T // KB        # 16
N_CC = C // 128       # contraction chunks over C = 8
N_GCB = GC // 128     # head pairs = 4


def build_program():
    nc = bacc.Bacc("TRN2", target_bir_lowering=False, debug=False)

    xT = nc.dram_tensor("xT", [N_TC, 128, N_CC, TC], BF16, kind="ExternalInput").ap()
    wq = nc.dram_tensor("wq", [128, N_GCB, N_CC, 128], BF16, kind="ExternalInput").ap()
    wk = nc.dram_tensor("wk", [128, N_GCB, N_CC, 128], BF16, kind="ExternalInput").ap()
    wv = nc.dram_tensor("wv", [128, N_CC, GC], BF16, kind="ExternalInput").ap()
    wo = nc.dram_tensor("wo", [128, N_GCB, C], BF16, kind="ExternalInput").ap()
    masks = nc.dram_tensor("masks", [KB, 2, KB], BF16, kind="ExternalInput").ap()
    sel_in = nc.dram_tensor("sel", [2, 128], BF16, kind="ExternalInput").ap()
    out = nc.dram_tensor("out", [T, C], BF16, kind="ExternalOutput").ap()

    EXP = mybir.ActivationFunctionType.Exp

    with tile.TileContext(nc) as tc:
        with (
            tc.tile_pool(name="persist", bufs=1) as pp,
            tc.tile_pool(name="xp", bufs=2) as xp,
            tc.tile_pool(name="pr_pool", bufs=4) as prp,
            tc.tile_pool(name="ot_pool", bufs=4) as otp,
            tc.tile_pool(name="dn_pool", bufs=2) as dnp,
            tc.tile_pool(name="sc_psum", bufs=2, space="PSUM") as scp,
            tc.tile_pool(name="av_psum", bufs=1, space="PSUM") as avp,
            tc.tile_pool(name="ps_psum", bufs=2, space="PSUM") as psp,
        ):
            qt = pp.tile([128, N_GCB, T], BF16)
            kt = pp.tile([128, N_GCB, T], BF16)
            vaug = pp.tile([128, N_KB, HPG, Dh + 1], BF16)
            aot = pp.tile([128, N_GCB, T], BF16)
            msk = pp.tile([KB, 2, KB], BF16)
            sel = pp.tile([2, 128], BF16)
            wqs = pp.tile([128, N_GCB, N_CC, 128], BF16)
            wks = pp.tile([128, N_GCB, N_CC, 128], BF16)
            wvs = pp.tile([128, N_CC, GC], BF16)
            wos = pp.tile([128, N_GCB, C], BF16)

            # ---- input DMAs: x chunk 0 on the scalar queue, weights on ---
            # ---- sync, so desc-gen and transfers overlap -----------------
            xts = [None] * N_TC

            def dma_x(t, eng=None):
                xts[t] = xp.tile([128, N_CC, TC], BF16, tag="xt", name=f"xt{t}")
                (eng or nc.sync).dma_start(xts[t][:], xT[t])

            # chunk-0 x split into eight independent slab tiles so the
            # first projection matmuls start as soon as the first slab lands
            x0q = []
            for q in range(N_CC):
                x0t = xp.tile([128, 1, TC], BF16, tag=f"x0q{q}", name=f"x0q{q}", bufs=1)
                x0q.append(x0t)
            nc.scalar.dma_start(wqs[:, 0, 0:2], wq[:, 0, 0:2])
            nc.scalar.dma_start(wqs[:, 0, 2:], wq[:, 0, 2:])
            for oc in range(1, N_GCB):
                nc.scalar.dma_start(wqs[:, oc], wq[:, oc])
            for q in range(N_CC):
                nc.sync.dma_start(x0q[q][:], xT[0][:, q:q + 1])
            for oc in range(N_GCB):
                nc.sync.dma_start(wks[:, oc], wk[:, oc])
            nc.sync.dma_start(wvs[:], wv)
            nc.sync.dma_start(msk[:], masks)
            nc.sync.dma_start(sel[:], sel_in)

            def xslice(t, kc):
                if t == 0:
                    return x0q[kc][:, 0]
                return xts[t][:, kc]
            nc.vector.memset(vaug[:, :, :, Dh:], 1.0)

            # ---- qkv projection work units for token chunk t -------------
            def unit_qk(t, oc, w_s, dst):
                ps = psp.tile([128, TC], F32, tag="ps", name="pjq")
                for kc in range(N_CC):
                    nc.tensor.matmul(
                        ps[:], w_s[:, oc, kc], xslice(t, kc),
                        start=(kc == 0), stop=(kc == N_CC - 1),
                    )
                nc.vector.tensor_copy(dst[:, oc, t * TC:(t + 1) * TC], ps[:])

            def unit_v(t, tb):
                ps = psp.tile([128, GC], F32, tag="ps", name="pjv")
                for kc in range(N_CC):
                    nc.tensor.matmul(
                        ps[:], xslice(t, kc)[:, tb * 128:(tb + 1) * 128],
                        wvs[:, kc],
                        start=(kc == 0), stop=(kc == N_CC - 1),
                    )
                nc.vector.tensor_copy(
                    vaug[:, t * 4 + tb, :, :Dh],
                    ps.rearrange("p (h d) -> p h d", h=HPG),
                )

            def phase2_units(t, q_first=False):
                us = []
                if q_first:
                    for oc in range(N_GCB):
                        us.append(lambda oc=oc: unit_qk(t, oc, wqs, qt))
                    for oc in range(N_GCB):
                        us.append(lambda oc=oc: unit_qk(t, oc, wks, kt))
                else:
                    for oc in range(N_GCB):
                        us.append(lambda oc=oc: unit_qk(t, oc, wqs, qt))
                        us.append(lambda oc=oc: unit_qk(t, oc, wks, kt))
                for tb in range(4):
                    us.append(lambda tb=tb: unit_v(t, tb))
                return us

            # ---- micro-op decompositions for fine-grained interleave -----
            def micro_qk(t, oc, w_s, dst):
                st = {}
                def a():
                    st["ps"] = psp.tile([128, TC], F32, tag="ps", name="pjq")
                    for kc in range(4):
                        nc.tensor.matmul(
                            st["ps"][:], w_s[:, oc, kc], xts[t][:, kc],
                            start=(kc == 0), stop=False,
                        )
                def b():
                    for kc in range(4, N_CC):
                        nc.tensor.matmul(
                            st["ps"][:], w_s[:, oc, kc], xts[t][:, kc],
                            start=False, stop=(kc == N_CC - 1),
                        )
                    nc.vector.tensor_copy(
                        dst[:, oc, t * TC:(t + 1) * TC], st["ps"][:]
                    )
                return [a, b]

            def micro_v(t, tb):
                st = {}
                def a():
                    st["ps"] = psp.tile([128, GC], F32, tag="ps", name="pjv")
                    for kc in range(4):
                        nc.tensor.matmul(
                            st["ps"][:],
                            xts[t][:, kc, tb * 128:(tb + 1) * 128],
                            wvs[:, kc], start=(kc == 0), stop=False,
                        )
                def b():
                    for kc in range(4, N_CC):
                        nc.tensor.matmul(
                            st["ps"][:],
                            xts[t][:, kc, tb * 128:(tb + 1) * 128],
                            wvs[:, kc], start=False, stop=(kc == N_CC - 1),
                        )
                    nc.vector.tensor_copy(
                        vaug[:, t * 4 + tb, :, :Dh],
                        st["ps"].rearrange("p (h d) -> p h d", h=HPG),
                    )
                return [a, b]

            def micro_tb(tb):
                st = {}
                def half(oc, lo):
                    if oc == 0 and lo == 0:
                        ots[tb % 4] = otp.tile([128, C], BF16, tag="ot", name="ot")
                    if lo == 0:
                        st["ps"] = psp.tile([128, TC], F32, tag="ps", name="op")
                    for cc in range(lo, lo + 2):
                        nc.tensor.matmul(
                            st["ps"][:],
                            aot[:, cc, tb * 128:(tb + 1) * 128],
                            wos[:, cc, oc * TC:(oc + 1) * TC],
                            start=(cc == 0), stop=(cc == N_GCB - 1),
                        )
                    if lo == 2:
                        nc.vector.tensor_copy(
                            ots[tb % 4][:, oc * TC:(oc + 1) * TC], st["ps"][:]
                        )
                        if oc == 1:
                            nc.sync.dma_start(
                                out[tb * 128:(tb + 1) * 128], ots[tb % 4][:]
                            )
                return [lambda oc=oc, lo=lo: half(oc, lo)
                        for oc in range(2) for lo in (0, 2)]

            ots = [None] * 4

            rec_t = [None] * (N_TC * N_GCB)
            from collections import deque
            dq = deque()   # (deadline (j,p,kb), fn) — deadlines non-decreasing
            fq = deque()   # free micros (no ordering constraint)

            def run_due(pos):
                while dq and dq[0][0] <= pos:
                    dq.popleft()[1]()

            def inject():
                if dq:
                    dq.popleft()[1]()
                elif fq:
                    fq.popleft()()

            # ---- attention + fused normalize for (chunk j, head pair p) --
            def attn_pair(j, p):
                av = avp.tile([Dh + 1, 2, TC], F32, tag="av", name="av")
                nkb = 4 * j + 4
                for kb in range(nkb):
                    run_due((j, p, kb))
                    off = KB * (kb - 4 * j) if kb >= 4 * j else 0
                    sc = scp.tile([128, 2, TC], F32, tag="sc", name="sc")
                    for half in range(2):
                        p0 = half * Dh
                        nc.tensor.matmul(
                            sc[:, half, off:],
                            kt[p0:p0 + Dh, p, kb * KB:(kb + 1) * KB],
                            qt[p0:p0 + Dh, p, j * TC + off:(j + 1) * TC],
                            start=True, stop=True,
                        )
                    pr = prp.tile([128, 2, TC], BF16, tag="pr", name="pr")
                    nc.scalar.activation(pr[:, :, off:], sc[:, :, off:], EXP)
                    if kb >= 4 * j:
                        nc.vector.tensor_mul(
                            pr[:, :, off:off + KB], pr[:, :, off:off + KB],
                            msk[:],
                        )
                    if kb % 6 == 1:
                        inject()
                    for half in range(2):
                        nc.tensor.matmul(
                            av[:, half, off:],
                            vaug[:, kb, 2 * p + half],
                            pr[:, half, off:],
                            start=(kb == 0), stop=(kb == nkb - 1),
                            skip_group_check=True,
                        )
                # denominators first (they head the recip->norm chain),
                # then the attn-output drains
                dn = dnp.tile([1, 2, TC], F32, tag="dn", name="dn", bufs=4)
                if (j, p) == (N_TC - 1, N_GCB - 1):
                    nc.scalar.copy(dn[:], av[Dh:Dh + 1])
                else:
                    nc.vector.tensor_copy(dn[:], av[Dh:Dh + 1])
                for half in range(2):
                    p0 = half * Dh
                    nc.vector.tensor_copy(
                        aot[p0:p0 + Dh, p, j * TC:(j + 1) * TC], av[:Dh, half]
                    )
                dc = dnp.tile([2, TC], F32, tag="dc", name="dc", bufs=4)
                nc.gpsimd.dma_start(dc[:], dn[0:1])
                rc = dnp.tile([2, TC], F32, tag="rc", name="rc", bufs=4)
                nc.vector.reciprocal_approx_fast(rc[:], dc[:])
                rcb = dnp.tile([2, TC], BF16, tag="rcb", name="rcb", bufs=4)
                rec_t[4 * j + p] = rcb
                nc.vector.tensor_copy(rcb[:], rc[:])

            # ---- softmax divide for (chunk j, head pair p), scheduled ----
            # ---- one pair later so the PE never waits on the recip chain -
            def norm_pair(j, p):
                rc = rec_t[4 * j + p]  # bf16 reciprocal rows
                bc = psp.tile([128, TC], F32, tag="ps", name="bc")
                nc.tensor.matmul(
                    bc[:], sel[:], rc[:],
                    start=True, stop=True,
                )
                nc.vector.tensor_mul(
                    aot[:, p, j * TC:(j + 1) * TC],
                    aot[:, p, j * TC:(j + 1) * TC],
                    bc[:],
                )

            # ---- out-projection for one token block ----------------------
            def tail_tb(tb):
                ot = otp.tile([128, C], BF16, tag="ot", name="ot")
                for oc in range(2):
                    ps = psp.tile([128, TC], F32, tag="ps", name="op")
                    for cc in range(N_GCB):
                        nc.tensor.matmul(
                            ps[:],
                            aot[:, cc, tb * 128:(tb + 1) * 128],
                            wos[:, cc, oc * TC:(oc + 1) * TC],
                            start=(cc == 0), stop=(cc == N_GCB - 1),
                        )
                    if oc == 0:
                        nc.scalar.copy(ot[:, oc * TC:(oc + 1) * TC], ps[:])
                    else:
                        nc.vector.tensor_copy(ot[:, oc * TC:(oc + 1) * TC], ps[:])
                    nc.sync.dma_start(
                        out[tb * 128:(tb + 1) * 128, oc * TC:(oc + 1) * TC],
                        ot[:, oc * TC:(oc + 1) * TC],
                    )

            # ---- interleaved schedule ------------------------------------
            for u in phase2_units(0, q_first=True):
                u()

            for j in range(N_TC):
                t = j + 1
                if t < N_TC:
                    dma_x(t)
                if j == 0:
                    nc.sync.dma_start(wos[:], wo)
                if j >= 1:
                    for tb in range(4 * (j - 1), 4 * j):
                        fq.extend(micro_tb(tb))
                if t < N_TC:
                    for m in micro_qk(t, 0, wqs, qt):
                        dq.append(((t, 0, 0), m))
                    for m in micro_qk(t, 0, wks, kt):
                        dq.append(((t, 0, 4 * t), m))
                    for tb in range(4):
                        for m in micro_v(t, tb):
                            dq.append(((t, 0, 4 * t + tb), m))
                    for oc in range(1, N_GCB):
                        for m in micro_qk(t, oc, wqs, qt):
                            dq.append(((t, oc, 0), m))
                        for m in micro_qk(t, oc, wks, kt):
                            dq.append(((t, oc, 4 * t), m))
                for p in range(N_GCB):
                    attn_pair(j, p)
                    if p >= 1:
                        norm_pair(j, p - 1)
                    inject()
                    inject()
                    inject()
                if j < N_TC - 1:
                    fq.append(lambda j=j: norm_pair(j, 3))
            while dq or fq:
                inject()
            # final chunk: out-projection cc0-2 partial chains overlap the
            # last attention pair's exp stream; only the cc3 matmuls (and
            # the cast+DMA) wait on the last pair's normalization
            last = N_TC - 1
            chains = []
            for k, tb in enumerate(range(4 * last, 4 * last + 2)):
                hold = scp.tile([128, 2, TC], F32, tag="sc", name="opsc")
                cps = [hold[:, 0], hold[:, 1]]
                for oc in range(2):
                    for cc in range(3):
                        nc.tensor.matmul(
                            cps[oc][:],
                            aot[:, cc, tb * 128:(tb + 1) * 128],
                            wos[:, cc, oc * TC:(oc + 1) * TC],
                            start=(cc == 0), stop=False,
                            skip_group_check=True,
                        )
                chains.append((tb, cps))
            norm_pair(last, 3)
            for tb, cps in chains:
                ot = otp.tile([128, C], BF16, tag="ot", name="ot")
                for oc in range(2):
                    nc.tensor.matmul(
                        cps[oc][:],
                        aot[:, 3, tb * 128:(tb + 1) * 128],
                        wos[:, 3, oc * TC:(oc + 1) * TC],
                        start=False, stop=True,
                        skip_group_check=True,
                    )
                    eng = nc.scalar if oc == 0 else nc.vector
                    if oc == 0:
                        nc.scalar.copy(ot[:, oc * TC:(oc + 1) * TC], cps[oc][:])
                    else:
                        nc.vector.tensor_copy(
                            ot[:, oc * TC:(oc + 1) * TC], cps[oc][:]
                        )
                    nc.sync.dma_start(
                        out[tb * 128:(tb + 1) * 128, oc * TC:(oc + 1) * TC],
                        ot[:, oc * TC:(oc + 1) * TC],
                    )
            tail_tb(4 * last + 2)
            tail_tb(4 * last + 3)

    nc.compile()
    return nc


_CACHE = {}


def _make_masks():
    m = np.zeros((KB, 2, KB), np.float32)
    for dk in range(KB):
        m[dk, :, dk:] = 1.0
    return m.astype(_BF)


def _make_sel():
    s = np.zeros((2, 128), np.float32)
    for m in range(128):
        s[m // Dh, m] = 1.0
    return s.astype(_BF)


def make_in_maps(x, W_qkv, W_out):
    masks = _make_masks()
    sel = _make_sel()
    in_maps = []
    for core in range(N_CORES):
        b, g = divmod(core, G)
        cs = slice(g * GC, (g + 1) * GC)
        xt_arr = np.ascontiguousarray(
            x[b].T.reshape(N_CC, 128, N_TC, TC).transpose(2, 1, 0, 3)
        ).astype(_BF)
        wq_l = np.ascontiguousarray(
            (W_qkv[:, cs] * 0.125)
            .reshape(N_CC, 128, N_GCB, 2, Dh)
            .transpose(1, 2, 0, 3, 4)
            .reshape(128, N_GCB, N_CC, 128)
        ).astype(_BF)
        wk_l = np.ascontiguousarray(
            W_qkv[:, C + g * GC:C + (g + 1) * GC]
            .reshape(N_CC, 128, N_GCB, 2, Dh)
            .transpose(1, 2, 0, 3, 4)
            .reshape(128, N_GCB, N_CC, 128)
        ).astype(_BF)
        wv_l = np.ascontiguousarray(
            W_qkv[:, 2 * C + g * GC:2 * C + (g + 1) * GC]
            .reshape(N_CC, 128, GC)
            .transpose(1, 0, 2)
        ).astype(_BF)
        wo_l = np.ascontiguousarray(
            W_out[cs, :]
            .reshape(N_GCB, 2, Dh, C)
            .transpose(1, 2, 0, 3)
            .reshape(128, N_GCB, C)
        ).astype(_BF)
        in_maps.append({
            "xT": xt_arr,
            "wq": wq_l,
            "wk": wk_l,
            "wv": wv_l,
            "wo": wo_l,
            "masks": masks,
            "sel": sel,
        })
    return in_maps


def kernel(x, W_qkv, W_out):
    x = np.ascontiguousarray(np.asarray(x, dtype=np.float32))
    W_qkv = np.asarray(W_qkv, dtype=np.float32)
    W_out = np.asarray(W_out, dtype=np.float32)

    if "nc" not in _CACHE:
        _CACHE["nc"] = build_program()
    nc = _CACHE["nc"]

    in_maps = make_in_maps(x, W_qkv, W_out)
    res = bass_utils.run_bass_kernel_spmd(nc, in_maps, core_ids=list(range(N_CORES)))

    out = np.empty((B, T, C), np.float32)
    for b in range(B):
        acc = res.results[G * b]["out"].astype(np.float32)
        for g in range(1, G):
            acc = acc + res.results[G * b + g]["out"].astype(np.float32)
        out[b] = acc
    return out


# revision 25
# speedup vs baseline: 1.0137x; 1.0090x over previous
"""Multi-head causal attention on 8 Trainium2 NeuronCores.

Sharding: data-parallel over batch (4) x tensor-parallel over heads (2 groups
of 8 heads). Each core computes a partial output [T, C] for one batch element
using its 8 heads; the host sums the two partials per batch element (the
"all-reduce after out_proj" done during unshard).

Design notes (HW exec ~275us vs 394us baseline):
  - Inputs host-pre-arranged so every DMA is contiguous per partition; the
    first-needed weights go on the scalar DMA queue and x chunk 0 is split
    into four independent quarter-tiles so the first matmul starts ~13us.
  - One interleaved instruction stream: projection work units for token
    chunk t+1, the out-projection for chunk j-1, and per-pair softmax
    normalization are emitted between (and sparsely inside) attention
    head-pairs of chunk j via a deadline-guarded micro-op queue, so the PE
    never idles long enough for the HAM clock gate to re-throttle. The
    deadline guard force-emits any deferred producer right before its
    consumer, so correctness never depends on the injection cadence.
  - Causal staircase computed at partial width: for key block kb of query
    chunk j only queries >= kb*128 are computed (saves ~25% of score/AV
    matmul columns and exp columns); only the leading 128 columns of a
    diagonal block need the triangular mask multiply.
  - Denominators (ones-row of the augmented V matmul) are staged through a
    1-partition tile, spread to 2 partitions by a tiny DMA on the otherwise
    idle GpSimd queue (keeping them off the busy sync queue), reciprocal'd
    with the 1-op ~51-ULP approx reciprocal, broadcast via a K=2 bf16
    matmul, and applied in-place to attn_outT one pair later so the PE
    never waits on the chain.
  - Final chunk's out-projection runs cc0-2 partial chains in the freed
    score-PSUM slots concurrently with the last attention pair; only the
    cc3 matmuls + cast + DMA trail the last normalization.
  - Output written bf16 (halves writeback); host upcasts and sums partials.

Per-core layouts (partition dim first):
  qt/kt/aot [128, 4, 2048]: partition = (head%2)*64 + d, dim1 = head//2 (pair)
  vaug [128, 16, 8, 65] bf16: partition = key-in-block, ones-augmented col 64
  scores^T per (pair, kb): psum [128, 2, 512] = key x (half, query)
"""

import numpy as np
import ml_dtypes

_BF = ml_dtypes.bfloat16

import concourse.bass as bass
import concourse.bacc as bacc
import concourse.mybir as mybir
import concourse.tile as tile
from concourse import bass_utils

F32 = mybir.dt.float32
F32R = mybir.dt.float32r
BF16 = mybir.dt.bfloat16

B, T, C = 4, 2048, 1024
H, Dh = 16, 64
G = 2                 # head groups (tensor parallel)
HPG = H // G          # 8 heads per group
GC = HPG * Dh         # group channels = 512
N_CORES = 8
TC = 512              # token chunk
KB = 128              # key block
N_TC = T // TC        # 4
N_KB = T // KB        # 16
N_CC = C // 128       # contraction chunks over C = 8
N_GCB = GC // 128     # head pairs = 4


def build_program():
    nc = bacc.Bacc("TRN2", target_bir_lowering=False, debug=False)

    xT = nc.dram_tensor("xT", [N_TC, 128, N_CC, TC], BF16, kind="ExternalInput").ap()
    wq = nc.dram_tensor("wq", [128, N_GCB, N_CC, 128], BF16, kind="ExternalInput").ap()
    wk = nc.dram_tensor("wk", [128, N_GCB, N_CC, 128], BF16, kind="ExternalInput").ap()
    wv = nc.dram_tensor("wv", [128, N_CC, GC], BF16, kind="ExternalInput").ap()
    wo = nc.dram_tensor("wo", [128, N_GCB, C], BF16, kind="ExternalInput").ap()
    masks = nc.dram_tensor("masks", [KB, 2, KB], BF16, kind="ExternalInput").ap()
    sel_in = nc.dram_tensor("sel", [2, 128], BF16, kind="ExternalInput").ap()
    out = nc.dram_tensor("out", [T, C], BF16, kind="ExternalOutput").ap()

    EXP = mybir.ActivationFunctionType.Exp

    with tile.TileContext(nc) as tc:
        with (
            tc.tile_pool(name="persist", bufs=1) as pp,
            tc.tile_pool(name="xp", bufs=2) as xp,
            tc.tile_pool(name="pr_pool", bufs=6) as prp,
            tc.tile_pool(name="ot_pool", bufs=4) as otp,
            tc.tile_pool(name="dn_pool", bufs=2) as dnp,
            tc.tile_pool(name="sc_psum", bufs=2, space="PSUM") as scp,
            tc.tile_pool(name="av_psum", bufs=1, space="PSUM") as avp,
            tc.tile_pool(name="ps_psum", bufs=2, space="PSUM") as psp,
        ):
            qt = pp.tile([128, N_GCB, T], BF16)
            kt = pp.tile([128, N_GCB, T], BF16)
            vaug = pp.tile([128, N_KB, HPG, Dh + 1], BF16)
            aot = pp.tile([128, N_GCB, T], BF16)
            msk = pp.tile([KB, 2, KB], BF16)
            sel = pp.tile([2, 128], BF16)
            wqs = pp.tile([128, N_GCB, N_CC, 128], BF16)
            wks = pp.tile([128, N_GCB, N_CC, 128], BF16)
            wvs = pp.tile([128, N_CC, GC], BF16)
            wos = pp.tile([128, N_GCB, C], BF16)

            # ---- input DMAs: x chunk 0 on the scalar queue, weights on ---
            # ---- sync, so desc-gen and transfers overlap -----------------
            xts = [None] * N_TC

            def dma_x(t, eng=None):
                xts[t] = xp.tile([128, N_CC, TC], BF16, tag="xt", name=f"xt{t}")
                (eng or nc.sync).dma_start(xts[t][:], xT[t])

            # chunk-0 x split into eight independent slab tiles so the
            # first projection matmuls start as soon as the first slab lands
            x0q = []
            for q in range(N_CC):
                x0t = xp.tile([128, 1, TC], BF16, tag=f"x0q{q}", name=f"x0q{q}", bufs=1)
                x0q.append(x0t)
            nc.scalar.dma_start(wqs[:, 0, 0:2], wq[:, 0, 0:2])
            nc.scalar.dma_start(wqs[:, 0, 2:], wq[:, 0, 2:])
            for oc in range(1, N_GCB):
                nc.scalar.dma_start(wqs[:, oc], wq[:, oc])
            for q in range(N_CC):
                nc.sync.dma_start(x0q[q][:], xT[0][:, q:q + 1])
            for oc in range(N_GCB):
                nc.sync.dma_start(wks[:, oc], wk[:, oc])
            nc.sync.dma_start(wvs[:], wv)
            nc.sync.dma_start(msk[:], masks)
            nc.sync.dma_start(sel[:], sel_in)

            def xslice(t, kc):
                if t == 0:
                    return x0q[kc][:, 0]
                return xts[t][:, kc]
            nc.vector.memset(vaug[:, :, :, Dh:], 1.0)

            # ---- qkv projection work units for token chunk t -------------
            def unit_qk(t, oc, w_s, dst):
                ps = psp.tile([128, TC], F32, tag="ps", name="pjq")
                for kc in range(N_CC):
                    nc.tensor.matmul(
                        ps[:], w_s[:, oc, kc], xslice(t, kc),
                        start=(kc == 0), stop=(kc == N_CC - 1),
                    )
                nc.vector.tensor_copy(dst[:, oc, t * TC:(t + 1) * TC], ps[:])

            def unit_v(t, tb):
                ps = psp.tile([128, GC], F32, tag="ps", name="pjv")
                for kc in range(N_CC):
                    nc.tensor.matmul(
                        ps[:], xslice(t, kc)[:, tb * 128:(tb + 1) * 128],
                        wvs[:, kc],
                        start=(kc == 0), stop=(kc == N_CC - 1),
                    )
                nc.vector.tensor_copy(
                    vaug[:, t * 4 + tb, :, :Dh],
                    ps.rearrange("p (h d) -> p h d", h=HPG),
                )

            def phase2_units(t, q_first=False):
                us = []
                if q_first:
                    for oc in range(N_GCB):
                        us.append(lambda oc=oc: unit_qk(t, oc, wqs, qt))
                    for oc in range(N_GCB):
                        us.append(lambda oc=oc: unit_qk(t, oc, wks, kt))
                else:
                    for oc in range(N_GCB):
                        us.append(lambda oc=oc: unit_qk(t, oc, wqs, qt))
                        us.append(lambda oc=oc: unit_qk(t, oc, wks, kt))
                for tb in range(4):
                    us.append(lambda tb=tb: unit_v(t, tb))
                return us

            # ---- micro-op decompositions for fine-grained interleave -----
            def micro_qk(t, oc, w_s, dst):
                st = {}
                def a():
                    st["ps"] = psp.tile([128, TC], F32, tag="ps", name="pjq")
                    for kc in range(4):
                        nc.tensor.matmul(
                            st["ps"][:], w_s[:, oc, kc], xts[t][:, kc],
                            start=(kc == 0), stop=False,
                        )
                def b():
                    for kc in range(4, N_CC):
                        nc.tensor.matmul(
                            st["ps"][:], w_s[:, oc, kc], xts[t][:, kc],
                            start=False, stop=(kc == N_CC - 1),
                        )
                    nc.vector.tensor_copy(
                        dst[:, oc, t * TC:(t + 1) * TC], st["ps"][:]
                    )
                return [a, b]

            def micro_v(t, tb):
                st = {}
                def a():
                    st["ps"] = psp.tile([128, GC], F32, tag="ps", name="pjv")
                    for kc in range(4):
                        nc.tensor.matmul(
                            st["ps"][:],
                            xts[t][:, kc, tb * 128:(tb + 1) * 128],
                            wvs[:, kc], start=(kc == 0), stop=False,
                        )
                def b():
                    for kc in range(4, N_CC):
                        nc.tensor.matmul(
                            st["ps"][:],
                            xts[t][:, kc, tb * 128:(tb + 1) * 128],
                            wvs[:, kc], start=False, stop=(kc == N_CC - 1),
                        )
                    nc.vector.tensor_copy(
                        vaug[:, t * 4 + tb, :, :Dh],
                        st["ps"].rearrange("p (h d) -> p h d", h=HPG),
                    )
                return [a, b]

            def micro_tb(tb):
                st = {}
                def half(oc, lo):
                    if oc == 0 and lo == 0:
                        ots[tb % 4] = otp.tile([128, C], BF16, tag="ot", name="ot")
                    if lo == 0:
                        st["ps"] = psp.tile([128, TC], F32, tag="ps", name="op")
                    for cc in range(lo, lo + 2):
                        nc.tensor.matmul(
                            st["ps"][:],
                            aot[:, cc, tb * 128:(tb + 1) * 128],
                            wos[:, cc, oc * TC:(oc + 1) * TC],
                            start=(cc == 0), stop=(cc == N_GCB - 1),
                        )
                    if lo == 2:
                        nc.vector.tensor_copy(
                            ots[tb % 4][:, oc * TC:(oc + 1) * TC], st["ps"][:]
                        )
                        if oc == 1:
                            nc.sync.dma_start(
                                out[tb * 128:(tb + 1) * 128], ots[tb % 4][:]
                            )
                return [lambda oc=oc, lo=lo: half(oc, lo)
                        for oc in range(2) for lo in (0, 2)]

            ots = [None] * 4

            rec_t = [None] * (N_TC * N_GCB)
            from collections import deque
            dq = deque()   # (deadline (j,p,kb), fn) — deadlines non-decreasing
            fq = deque()   # free micros (no ordering constraint)

            def run_due(pos):
                while dq and dq[0][0] <= pos:
                    dq.popleft()[1]()

            def inject():
                if dq:
                    dq.popleft()[1]()
                elif fq:
                    fq.popleft()()

            # ---- attention + fused normalize for (chunk j, head pair p) --
            def attn_pair(j, p):
                av = avp.tile([Dh + 1, 2, TC], F32, tag="av", name="av")
                nkb = 4 * j + 4
                for kb in range(nkb):
                    run_due((j, p, kb))
                    off = KB * (kb - 4 * j) if kb >= 4 * j else 0
                    sc = scp.tile([128, 2, TC], F32, tag="sc", name="sc")
                    for half in range(2):
                        p0 = half * Dh
                        nc.tensor.matmul(
                            sc[:, half, off:],
                            kt[p0:p0 + Dh, p, kb * KB:(kb + 1) * KB],
                            qt[p0:p0 + Dh, p, j * TC + off:(j + 1) * TC],
                            start=True, stop=True,
                        )
                    pr = prp.tile([128, 2, TC], BF16, tag="pr", name="pr")
                    nc.scalar.activation(pr[:, :, off:], sc[:, :, off:], EXP)
                    if kb >= 4 * j:
                        nc.vector.tensor_mul(
                            pr[:, :, off:off + KB], pr[:, :, off:off + KB],
                            msk[:],
                        )
                    if kb % 6 == 1:
                        inject()
                    for half in range(2):
                        nc.tensor.matmul(
                            av[:, half, off:],
                            vaug[:, kb, 2 * p + half],
                            pr[:, half, off:],
                            start=(kb == 0), stop=(kb == nkb - 1),
                            skip_group_check=True,
                        )
                # denominators first (they head the recip->norm chain),
                # then the attn-output drains
                dn = dnp.tile([1, 2, TC], F32, tag="dn", name="dn", bufs=4)
                if j < N_TC - 1 or p == N_GCB - 1:
                    nc.scalar.copy(dn[:], av[Dh:Dh + 1])
                else:
                    nc.vector.tensor_copy(dn[:], av[Dh:Dh + 1])
                for half in range(2):
                    p0 = half * Dh
                    nc.vector.tensor_copy(
                        aot[p0:p0 + Dh, p, j * TC:(j + 1) * TC], av[:Dh, half]
                    )
                dc = dnp.tile([2, TC], F32, tag="dc", name="dc", bufs=4)
                nc.gpsimd.dma_start(dc[:], dn[0:1])
                rc = dnp.tile([2, TC], F32, tag="rc", name="rc", bufs=4)
                nc.vector.reciprocal_approx_fast(rc[:], dc[:])
                rcb = dnp.tile([2, TC], BF16, tag="rcb", name="rcb", bufs=4)
                rec_t[4 * j + p] = rcb
                nc.vector.tensor_copy(rcb[:], rc[:])

            # ---- softmax divide for (chunk j, head pair p), scheduled ----
            # ---- one pair later so the PE never waits on the recip chain -
            def norm_pair(j, p):
                rc = rec_t[4 * j + p]  # bf16 reciprocal rows
                bc = psp.tile([128, TC], F32, tag="ps", name="bc")
                nc.tensor.matmul(
                    bc[:], sel[:], rc[:],
                    start=True, stop=True,
                )
                nc.vector.tensor_mul(
                    aot[:, p, j * TC:(j + 1) * TC],
                    aot[:, p, j * TC:(j + 1) * TC],
                    bc[:],
                )

            # ---- out-projection for one token block ----------------------
            def tail_tb(tb):
                ot = otp.tile([128, C], BF16, tag="ot", name="ot")
                for oc in range(2):
                    ps = psp.tile([128, TC], F32, tag="ps", name="op")
                    for cc in range(N_GCB):
                        nc.tensor.matmul(
                            ps[:],
                            aot[:, cc, tb * 128:(tb + 1) * 128],
                            wos[:, cc, oc * TC:(oc + 1) * TC],
                            start=(cc == 0), stop=(cc == N_GCB - 1),
                        )
                    if oc == 0:
                        nc.scalar.copy(ot[:, oc * TC:(oc + 1) * TC], ps[:])
                    else:
                        nc.vector.tensor_copy(ot[:, oc * TC:(oc + 1) * TC], ps[:])
                    nc.sync.dma_start(
                        out[tb * 128:(tb + 1) * 128, oc * TC:(oc + 1) * TC],
                        ot[:, oc * TC:(oc + 1) * TC],
                    )

            # ---- interleaved schedule ------------------------------------
            for u in phase2_units(0, q_first=True):
                u()

            for j in range(N_TC):
                t = j + 1
                if t < N_TC:
                    dma_x(t)
                if j == 0:
                    nc.sync.dma_start(wos[:], wo)
                if j >= 1:
                    for tb in range(4 * (j - 1), 4 * j):
                        fq.extend(micro_tb(tb))
                if t < N_TC:
                    for m in micro_qk(t, 0, wqs, qt):
                        dq.append(((t, 0, 0), m))
                    for m in micro_qk(t, 0, wks, kt):
                        dq.append(((t, 0, 4 * t), m))
                    for tb in range(4):
                        for m in micro_v(t, tb):
                            dq.append(((t, 0, 4 * t + tb), m))
                    for oc in range(1, N_GCB):
                        for m in micro_qk(t, oc, wqs, qt):
                            dq.append(((t, oc, 0), m))
                        for m in micro_qk(t, oc, wks, kt):
                            dq.append(((t, oc, 4 * t), m))
                for p in range(N_GCB):
                    attn_pair(j, p)
                    if p >= 1:
                        norm_pair(j, p - 1)
                    inject()
                    inject()
                    inject()
                if j < N_TC - 1:
                    fq.append(lambda j=j: norm_pair(j, 3))
            while dq or fq:
                inject()
            # final chunk: out-projection cc0-2 partial chains overlap the
            # last attention pair's exp stream; only the cc3 matmuls (and
            # the cast+DMA) wait on the last pair's normalization
            last = N_TC - 1
            chains = []
            for k, tb in enumerate(range(4 * last, 4 * last + 2)):
                hold = scp.tile([128, 2, TC], F32, tag="sc", name="opsc")
                cps = [hold[:, 0], hold[:, 1]]
                for oc in range(2):
                    for cc in range(3):
                        nc.tensor.matmul(
                            cps[oc][:],
                            aot[:, cc, tb * 128:(tb + 1) * 128],
                            wos[:, cc, oc * TC:(oc + 1) * TC],
                            start=(cc == 0), stop=False,
                            skip_group_check=True,
                        )
                chains.append((tb, cps))
            norm_pair(last, 3)
            for tb, cps in chains:
                ot = otp.tile([128, C], BF16, tag="ot", name="ot")
                for oc in range(2):
                    nc.tensor.matmul(
                        cps[oc][:],
                        aot[:, 3, tb * 128:(tb + 1) * 128],
                        wos[:, 3, oc * TC:(oc + 1) * TC],
                        start=False, stop=True,
                        skip_group_check=True,
                    )
                    eng = nc.scalar if oc == 0 else nc.vector
                    if oc == 0:
                        nc.scalar.copy(ot[:, oc * TC:(oc + 1) * TC], cps[oc][:])
                    else:
                        nc.vector.tensor_copy(
                            ot[:, oc * TC:(oc + 1) * TC], cps[oc][:]
                        )
                    nc.sync.dma_start(
                        out[tb * 128:(tb + 1) * 128, oc * TC:(oc + 1) * TC],
                        ot[:, oc * TC:(oc + 1) * TC],
                    )
            tail_tb(4 * last + 2)
            tail_tb(4 * last + 3)

    nc.compile()
    return nc


_CACHE = {}


def _make_masks():
    m = np.zeros((KB, 2, KB), np.float32)
    for dk in range(KB):
        m[dk, :, dk:] = 1.0
    return m.astype(_BF)


def _make_sel():
    s = np.zeros((2, 128), np.float32)
    for m in range(128):
        s[m // Dh, m] = 1.0
    return s.astype(_BF)


def make_in_maps(x, W_qkv, W_out):
    masks = _make_masks()
    sel = _make_sel()
    in_maps = []
    for core in range(N_CORES):
        b, g = divmod(core, G)
        cs = slice(g * GC, (g + 1) * GC)
        xt_arr = np.ascontiguousarray(
            x[b].T.reshape(N_CC, 128, N_TC, TC).transpose(2, 1, 0, 3)
        ).astype(_BF)
        wq_l = np.ascontiguousarray(
            (W_qkv[:, cs] * 0.125)
            .reshape(N_CC, 128, N_GCB, 2, Dh)
            .transpose(1, 2, 0, 3, 4)
            .reshape(128, N_GCB, N_CC, 128)
        ).astype(_BF)
        wk_l = np.ascontiguousarray(
            W_qkv[:, C + g * GC:C + (g + 1) * GC]
            .reshape(N_CC, 128, N_GCB, 2, Dh)
            .transpose(1, 2, 0, 3, 4)
            .reshape(128, N_GCB, N_CC, 128)
        ).astype(_BF)
        wv_l = np.ascontiguousarray(
            W_qkv[:, 2 * C + g * GC:2 * C + (g + 1) * GC]
            .reshape(N_CC, 128, GC)
            .transpose(1, 0, 2)
        ).astype(_BF)
        wo_l = np.ascontiguousarray(
            W_out[cs, :]
            .reshape(N_GCB, 2, Dh, C)
            .transpose(1, 2, 0, 3)
            .reshape(128, N_GCB, C)
        ).astype(_BF)
        in_maps.append({
            "xT": xt_arr,
            "wq": wq_l,
            "wk": wk_l,
            "wv": wv_l,
            "wo": wo_l,
            "masks": masks,
            "sel": sel,
        })
    return in_maps


def kernel(x, W_qkv, W_out):
    x = np.ascontiguousarray(np.asarray(x, dtype=np.float32))
    W_qkv = np.asarray(W_qkv, dtype=np.float32)
    W_out = np.asarray(W_out, dtype=np.float32)

    if "nc" not in _CACHE:
        _CACHE["nc"] = build_program()
    nc = _CACHE["nc"]

    in_maps = make_in_maps(x, W_qkv, W_out)
    res = bass_utils.run_bass_kernel_spmd(nc, in_maps, core_ids=list(range(N_CORES)))

    out = np.empty((B, T, C), np.float32)
    for b in range(B):
        acc = res.results[G * b]["out"].astype(np.float32)
        for g in range(1, G):
            acc = acc + res.results[G * b + g]["out"].astype(np.float32)
        out[b] = acc
    return out
